# revision 6
# baseline (speedup 1.0000x reference)
"""Trainium2 Bass kernel for an autoregressive-flow (MAF) layer.

Reference computation (per region r, batch-network b):
    xr[n, d]   = x[n, region_idx[r, d]]                      # [N, D]
    h1 = relu(xr @ (W1*M1)[r,b])                             # [N, H]
    h2 = relu(h1 @ (W2*M2)[r,b])                             # [N, H]
    o  = h2 @ (W3*M3)[r,b]                                   # [N, 2D]
    shift = o[:, 0::2]; log_scale = o[:, 1::2]
    u  = (xr - shift) * exp(-log_scale)
    ll[n, r, b] = sum_d(-0.5*u^2 - 0.5*log(2*pi) - log_scale)

Sharding: region axis R=8 across the 8 NeuronCores; each core handles its
region's B=16 networks over all N=2048 samples.

Device dataflow (per core, "transposed" orientation, 4 chunks of 512):
    - xtb [128, 2048] bf16: x-slice transposed, replicated on 4 partition
      row-groups. DMA'd in two pieces so chunk-0 compute starts early.
    - All weights+masks packed per group-of-4-nets into one [128, 2, 896]
      DMA (w||m); masked weights = one bf16 multiply per group.
    - h1/h2 PSUM tiles are [128, 2, 512] fp32 PAIRS (two nets, two banks);
      one relu instruction drains a whole pair (halves the per-op overhead
      on ACT/DVE, which are the bottleneck engines). A single rotating
      2-pair PSUM pool is reused L1 -> L2 within each group.
    - L1: 4 row-tiled K=32 matmuls issued back-to-back (PE-array row
      tiling runs them concurrently). L3: per-bank 4 col-tiled M=32
      matmuls. shift bank seeded with -x via a negated tiled identity.
    - Tail per group: A = 0.5*(s-x)^2 [ACT Square, 0.5 folded into the
      scale], B = exp(-2*ls) [ACT], t1 = A*B [GpSimd, SBUF-only], and
      c = t1 + ls [DVE, fused PSUM read] so that
      ll = -sum_d(c) - D*0.5*log(2pi): a single block(-1)-weights matmul
      per group accumulates the whole log-likelihood (no separate
      log_scale copy or second reduction matmul).
"""

import math

import ml_dtypes
import numpy as np

import concourse.bacc as bacc
import concourse.mybir as mybir
from concourse.bass_utils import run_bass_kernel_spmd
from concourse.tile import TileContext

R, B, D, H, N, F = 8, 16, 32, 128, 2048, 256
HALF_LOG_2PI = 0.9189385332046727
N_CORES = 8
CHUNK = 512
F32 = mybir.dt.float32
F32R = mybir.dt.float32r
BF16 = mybir.dt.bfloat16

# Per-group packed weight row: w1 [32x128 rows] | w2 4x[128] | w3 4x[2x32]
W1_OFF, W2_OFF, W3_OFF, WROW = 0, 128, 640, 896


def _consts():
    # Negated tiled identity: out[m, n] = -xt[m % 32, n] when used as lhsT
    # against rhs = xt[0:32, :].
    neg_i4 = np.zeros((D, 128), np.float32)
    for m in range(128):
        neg_i4[m % D, m] = -1.0
    # Block reduction weights [128, 4 groups, 16 nets]: for group g,
    # column j = 4g+bp sums partition rows 32bp..32bp+31 with weight -1.
    llw = np.zeros((128, 4, 16), np.float32)
    for g in range(4):
        for bp in range(4):
            llw[32 * bp : 32 * (bp + 1), g, 4 * g + bp] = -1.0
    return neg_i4, llw


def build_nc(n_total=N):
    assert n_total % CHUNK == 0
    n_chunks = n_total // CHUNK

    nc = bacc.Bacc(
        "TRN2",
        target_bir_lowering=False,
        debug=False,
        enable_asserts=False,
        num_devices=N_CORES,
    )

    xt4_d = nc.declare_dram_parameter("xt4", [128, n_total], BF16, isOutput=False)
    wm_d = nc.declare_dram_parameter("wm", [128, 4, 2, WROW], BF16, isOutput=False)
    out_d = nc.declare_dram_parameter("out", [n_chunks, 16, CHUNK], F32, isOutput=True)

    neg_i4_np, llw_np = _consts()
    neg_i4_d = nc.inline_tensor(neg_i4_np.astype(ml_dtypes.bfloat16), "neg_i4")
    llw_d = nc.inline_tensor(llw_np, "llw")

    with TileContext(nc) as tc:
        with (
            tc.tile_pool(name="const", bufs=1) as cpool,
            tc.tile_pool(name="wload", bufs=2) as lpool,
            tc.tile_pool(name="act", bufs=3) as apool,
            tc.tile_pool(name="tail", bufs=2) as tpool,
            tc.tile_pool(name="pp", bufs=2, space="PSUM") as pppool,
            tc.tile_pool(name="pt", bufs=1, space="PSUM") as ptpool,
            tc.tile_pool(name="pl", bufs=2, space="PSUM") as plpool,
            tc.tile_pool(name="pll", bufs=1, space="PSUM") as pllpool,
        ):
            xtb = cpool.tile([128, n_total], BF16, tag="xtb")
            neg_i4 = cpool.tile([D, 128], BF16, tag="negi4")
            llw = cpool.tile([128, 4, 16], F32R, tag="llw")
            # Chunk-0 columns first so compute starts before the full x
            # transfer lands; weights stream in parallel on gpsimd's queue.
            nc.sync.dma_start(out=xtb[:, 0:CHUNK], in_=xt4_d[:, 0:CHUNK])
            nc.sync.dma_start(out=neg_i4[:], in_=neg_i4_d[:])
            nc.sync.dma_start(out=xtb[:, CHUNK:], in_=xt4_d[:, CHUNK:])
            llwstage = lpool.tile([128, 4, 16], F32, tag="llwf")
            nc.sync.dma_start(out=llwstage[:], in_=llw_d[:])
            nc.vector.tensor_copy(out=llw[:], in_=llwstage[:])

            # Masked weights, computed once and kept resident. One DMA +
            # one bf16 multiply per group-of-4-networks.
            wall = cpool.tile([128, 4, WROW], BF16, tag="wall")
            for g in range(4):
                wmraw = lpool.tile([128, 2, WROW], BF16, tag="wm")
                nc.gpsimd.dma_start(out=wmraw[:], in_=wm_d[:, g, :, :])
                nc.vector.tensor_mul(
                    out=wall[:, g, :], in0=wmraw[:, 0], in1=wmraw[:, 1]
                )

            def w1(g, bp):  # [32, 128] lhsT for net 4g+bp (K=32 rows)
                return wall[32 * bp : 32 * (bp + 1), g, W1_OFF : W1_OFF + 128]

            def w2(g, bp):  # [128, 128] lhsT
                o = W2_OFF + 128 * bp
                return wall[:, g, o : o + 128]

            def w3(g, bp, half):  # [128, 32] lhsT (half 0=shift, 1=log_scale)
                o = W3_OFF + 64 * bp + 32 * half
                return wall[:, g, o : o + 32]

            sq_scale = float(math.sqrt(0.5))

            for c in range(n_chunks):
                cs = slice(c * CHUNK, (c + 1) * CHUNK)
                llps = pllpool.tile([16, CHUNK], F32, tag="llps")
                for g in range(4):
                    # ---- L1: 4 row-tiled K=32 matmuls into two bank-pairs.
                    p1 = []
                    for half in range(2):
                        p1.append(
                            pppool.tile([128, 2, CHUNK], F32, tag="pp", name="p1")
                        )
                    for bp in range(4):
                        prow = slice(32 * bp, 32 * (bp + 1))
                        nc.tensor.matmul(
                            p1[bp // 2][:, bp % 2, :],
                            w1(g, bp),
                            xtb[prow, cs],
                            start=True,
                            stop=True,
                            tile_position=(32 * bp, 0),
                        )
                    s1 = []
                    for half in range(2):
                        s = apool.tile([128, 2, CHUNK], BF16, tag="s1")
                        # Relu engine split tuned so ACT (which also owns
                        # Square+Exp) and DVE (fused add) finish together.
                        if half == 0:
                            nc.scalar.activation(
                                s[:], p1[half][:], mybir.ActivationFunctionType.Relu
                            )
                        else:
                            nc.vector.tensor_scalar_max(s[:], p1[half][:], 0.0)
                        s1.append(s)

                    # ---- L2: 4 full matmuls, reusing the same bank-pairs.
                    p2 = []
                    for half in range(2):
                        p2.append(
                            pppool.tile([128, 2, CHUNK], F32, tag="pp", name="p2")
                        )
                    for bp in range(4):
                        nc.tensor.matmul(
                            p2[bp // 2][:, bp % 2, :],
                            w2(g, bp),
                            s1[bp // 2][:, bp % 2, :],
                            start=True,
                            stop=True,
                        )
                    s2 = []
                    for half in range(2):
                        s = apool.tile([128, 2, CHUNK], BF16, tag="s2")
                        if half == 0 and g != 3:
                            nc.scalar.activation(
                                s[:], p2[half][:], mybir.ActivationFunctionType.Relu
                            )
                        else:
                            nc.vector.tensor_scalar_max(s[:], p2[half][:], 0.0)
                        s2.append(s)

                    # ---- L3 shift: T = shift - x in PSUM (identity seed,
                    # then 4 col-tiled M=32 strips).
                    tps = ptpool.tile([128, CHUNK], F32, tag="tps")
                    nc.tensor.matmul(
                        tps[:],
                        neg_i4[:],
                        xtb[0:D, cs],
                        start=True,
                        stop=False,
                        skip_group_check=True,
                        tile_position=(0, 0),
                    )
                    for bp in range(4):
                        prow = slice(32 * bp, 32 * (bp + 1))
                        nc.tensor.matmul(
                            tps[prow, :],
                            w3(g, bp, 0),
                            s2[bp // 2][:, bp % 2, :],
                            start=False,
                            stop=(bp == 3),
                            skip_group_check=True,
                            tile_position=(0, 32 * bp),
                        )
                    # ---- L3 log_scale: 4 col-tiled strips.
                    lps = plpool.tile([128, CHUNK], F32, tag="lps")
                    for bp in range(4):
                        prow = slice(32 * bp, 32 * (bp + 1))
                        nc.tensor.matmul(
                            lps[prow, :],
                            w3(g, bp, 1),
                            s2[bp // 2][:, bp % 2, :],
                            start=True,
                            stop=True,
                            tile_position=(0, 32 * bp),
                        )

                    # ---- tail: c = 0.5*u^2 + ls, summed with -1 weights.
                    a_sb = tpool.tile([128, CHUNK], F32, tag="a")
                    nc.scalar.activation(
                        a_sb[:],
                        tps[:],
                        mybir.ActivationFunctionType.Square,
                        scale=sq_scale,
                    )
                    b_sb = tpool.tile([128, CHUNK], F32, tag="b")
                    nc.scalar.activation(
                        b_sb[:], lps[:], mybir.ActivationFunctionType.Exp, scale=-2.0
                    )
                    t1_sb = tpool.tile([128, CHUNK], F32, tag="t1")
                    nc.gpsimd.tensor_mul(out=t1_sb[:], in0=a_sb[:], in1=b_sb[:])
                    c_sb = tpool.tile([128, CHUNK], F32R, tag="c")
                    nc.vector.tensor_add(out=c_sb[:], in0=t1_sb[:], in1=lps[:])

                    nc.tensor.matmul(
                        llps[:],
                        llw[:, g, :],
                        c_sb[:],
                        start=(g == 0),
                        stop=(g == 3),
                        skip_group_check=True,
                    )

                ll_sb = tpool.tile([16, CHUNK], F32, tag="ll")
                nc.vector.tensor_scalar_add(
                    ll_sb[:], llps[:], float(-D * HALF_LOG_2PI)
                )
                nc.sync.dma_start(out=out_d[c], in_=ll_sb[:])

    nc.compile()
    return nc


def shard_inputs(x, W1, W2, W3, M1, M2, M3, region_idx, n_total=N):
    """Per-core input dicts: pure gather/transpose/replicate layout prep."""
    x = np.asarray(x, dtype=np.float32)
    region_idx = np.asarray(region_idx)
    in_maps = []
    for r in range(N_CORES):
        xr = x[:n_total, region_idx[r]]  # [n, D]
        xt = np.ascontiguousarray(xr.T)  # [D, n]
        xt4 = np.ascontiguousarray(np.tile(xt, (4, 1)))  # [128, n]

        def prep1(w):
            w = np.asarray(w[r], dtype=np.float32)  # [16, 32, 128]
            return w.reshape(4, 4, D, H).transpose(1, 2, 0, 3).reshape(128, 4, H)

        def prep2(w):
            w = np.asarray(w[r], dtype=np.float32)  # [16, 128, 128]
            # [128, 4g, 4bp, 128] -> rows k, group, net, h
            return w.reshape(4, 4, H, H).transpose(2, 0, 1, 3).reshape(128, 4, 512)

        def prep3(w):
            w = np.asarray(w[r], dtype=np.float32)  # [16, 128, 64]
            # [k, g, bp, half, d]: out column order bp-major then half
            return (
                w.reshape(4, 4, H, D, 2)
                .transpose(2, 0, 1, 4, 3)
                .reshape(128, 4, 256)
            )

        def pack(a, b, c3):
            return np.concatenate([a, b, c3], axis=-1)  # [128, 4, 896]

        wrow = pack(prep1(W1), prep2(W2), prep3(W3))
        mrow = pack(prep1(M1), prep2(M2), prep3(M3))
        wm = np.stack([wrow, mrow], axis=2)  # [128, 4, 2, 896]

        in_maps.append(
            {
                "xt4": xt4.astype(ml_dtypes.bfloat16),
                "wm": np.ascontiguousarray(wm).astype(ml_dtypes.bfloat16),
            }
        )
    return in_maps


_NC_CACHE = {}


def run(x, W1, W2, W3, M1, M2, M3, region_idx, trace=False, n_total=N):
    if n_total not in _NC_CACHE:
        _NC_CACHE[n_total] = build_nc(n_total)
    nc = _NC_CACHE[n_total]
    in_maps = shard_inputs(x, W1, W2, W3, M1, M2, M3, region_idx, n_total)
    res = run_bass_kernel_spmd(
        nc, in_maps, core_ids=list(range(N_CORES)), trace=trace
    )
    out = np.empty((n_total, R, B), dtype=np.float32)
    for r in range(N_CORES):
        o = res.results[r]["out"]  # [n_chunks, 16, CHUNK]
        out[:, r, :] = o.transpose(0, 2, 1).reshape(n_total, B)
    return out, res


def kernel(x, W1, W2, W3, M1, M2, M3, region_idx):
    out, _ = run(x, W1, W2, W3, M1, M2, M3, region_idx)
    return out
